# revision 1
# baseline (speedup 1.0000x reference)
"""Trainium2 Bass kernel for nn_GRU_90426241450185.

Pipeline (3 SPMD launches over 8 NeuronCores):
  L1 (batch-parallel): per-core transpose of x + input projection GEMM,
     written as projT [4*D_STATE, S] per batch.
  L2 (head-parallel, 2 heads/core): fixed-point Jacobi sweeps over the GRU
     recurrence. Gate pre-activations come from f32r matmuls (x injected into
     PSUM via an identity matmul, recurrent term via block-diagonal weights);
     the state update h = f*h + (1-f)*c is re-solved exactly per sweep with
     the DVE's tensor_tensor_scan. Chunks of 512 timesteps are processed
     Gauss-Seidel style; 5 Jacobi sweeps per chunk converge to fp32-level.
  L3 (batch-parallel): y = h * silu(g), rmsnorm (norm_weight folded into
     w_out), output projection GEMM, transpose back to [S, D_OUT].

Precision: big GEMMs run as 3-term bf16 hi/lo splits (hi*hi + hi*lo + lo*hi);
recurrence matmuls run in f32r (hardware bf16-pair). End-to-end ~1.6e-5 rel.
"""

import numpy as np
import ml_dtypes

import bass_rust
import concourse.bass as bass
import concourse.mybir as mybir
from concourse import bacc
from concourse.bass_utils import run_bass_kernel_spmd
from concourse.tile import TileContext
from concourse.masks import make_identity
from concourse.vector_clock import ScopedClock

F32 = mybir.dt.float32
F32R = mybir.dt.float32r
BF16 = mybir.dt.bfloat16
AF = mybir.ActivationFunctionType
ALU = mybir.AluOpType

B, S = 8, 2048
D_IN, D_STATE, D_OUT = 1024, 1024, 1024
H, DH = 16, 64
EPS = 1e-6
N_CORES = 8

L1_TERMS = 3          # 3 = bf16 hi/lo 3-term GEMM, 1 = f32r single
L3_TERMS = 3
N_SWEEPS = 5
TC = 512              # L2 time-chunk length


# --- workaround: this walrus build accepts at most ~2 sem waits per
# instruction; fan the final TileContext drain's waits out across
# single-wait NOPs so the drain itself needs none.
def _patched_drain_and_barrier(self, tick_clock, wait_clock):
    gc = tick_clock.global_clock
    observed = bass_rust.VectorClock()
    for proc in range(64):
        try:
            t = gc.peek_next(proc) - 1
        except Exception:
            break
        if t <= 0:
            continue
        vc = bass_rust.VectorClock()
        vc.require_at_least(proc, t)
        nop = self.nc.sync.nop(nofuse=True)
        wait_clock.add_sem_waits(
            nop.ins, ScopedClock({None: vc}), ScopedClock({None: observed.copy()})
        )
        observed.require_at_least(proc, t)
    drain_inst = self.nc.sync.drain()
    wait_clock.add_sem_waits(
        drain_inst.ins, ScopedClock({None: gc}), ScopedClock({None: observed.copy()})
    )
    self.nc.all_engine_barrier()
    assert self.sems is not None
    popped = self.nc._tile_sem_poison_stack.pop()
    assert popped is self._sem_poison
    self.nc.clear_and_free_semaphores(list(self.sems.allocated().values()))
    self.nc.all_engine_barrier()


TileContext._drain_and_barrier = _patched_drain_and_barrier


def _bf16(a):
    return np.asarray(a).astype(ml_dtypes.bfloat16)


def _bf16_split(a):
    hi = _bf16(a)
    lo = _bf16(np.asarray(a, np.float32) - hi.astype(np.float32))
    return hi, lo


def _f32r_round(a):
    hi, lo = _bf16_split(a)
    return (hi.astype(np.float32) + lo.astype(np.float32)).astype(np.float32)


# ---------------------------------------------------------------- L1
def build_l1():
    nc = bacc.Bacc(name="gru_l1")
    x_d = nc.dram_tensor("x", [S, D_IN], F32, kind="ExternalInput")
    if L1_TERMS == 3:
        whi_d = nc.dram_tensor("whi", [D_IN, 4 * D_STATE], BF16, kind="ExternalInput")
        wlo_d = nc.dram_tensor("wlo", [D_IN, 4 * D_STATE], BF16, kind="ExternalInput")
    else:
        wr_d = nc.dram_tensor("wr", [D_IN, 4 * D_STATE], F32, kind="ExternalInput")
    pT_d = nc.dram_tensor("projT", [4 * D_STATE, S], F32, kind="ExternalOutput")

    KT = D_IN // 128        # 8 k tiles
    MT = (4 * D_STATE) // 128  # 32 m tiles
    NT = S // 512           # 4 n chunks
    TT = S // 128           # 16 token tiles

    with TileContext(nc) as tc:
        with tc.tile_pool(name="const", bufs=1) as cpool, \
             tc.tile_pool(name="xin", bufs=3) as xpool, \
             tc.tile_pool(name="xT", bufs=1) as xtpool, \
             tc.tile_pool(name="w", bufs=2) as wpool, \
             tc.tile_pool(name="ev", bufs=3) as evpool, \
             tc.tile_pool(name="pt", bufs=2, space="PSUM") as ptpool, \
             tc.tile_pool(name="pg", bufs=2, space="PSUM") as pgpool:

            ident = cpool.tile([128, 128], F32)
            make_identity(nc, ident[:])

            if L1_TERMS == 3:
                xThi = [xtpool.tile([128, S], BF16, tag=f"xthi{k}") for k in range(KT)]
                xTlo = [xtpool.tile([128, S], BF16, tag=f"xtlo{k}") for k in range(KT)]
            else:
                xTr = [xtpool.tile([128, S], F32R, tag=f"xtr{k}") for k in range(KT)]

            # build xT via PE transposes
            for tt in range(TT):
                xt = xpool.tile([128, D_IN], F32, tag="x")
                nc.sync.dma_start(out=xt[:], in_=x_d[tt * 128:(tt + 1) * 128, :])
                for kt in range(KT):
                    pt = ptpool.tile([128, 128], F32, tag="pt")
                    nc.tensor.transpose(pt[:], xt[:, kt * 128:(kt + 1) * 128], ident[:])
                    tsl = slice(tt * 128, (tt + 1) * 128)
                    if L1_TERMS == 3:
                        nc.vector.tensor_copy(xThi[kt][:, tsl], pt[:])
                        nc.vector.tensor_sub(xTlo[kt][:, tsl], pt[:], xThi[kt][:, tsl])
                    else:
                        nc.vector.tensor_copy(xTr[kt][:, tsl], pt[:])

            # GEMM
            for m in range(MT):
                msl = slice(m * 128, (m + 1) * 128)
                if L1_TERMS == 3:
                    whi = wpool.tile([128, KT, 128], BF16, tag="whi")
                    wlo = wpool.tile([128, KT, 128], BF16, tag="wlo")
                    nc.sync.dma_start(
                        out=whi[:],
                        in_=whi_d.rearrange("(kt p) m -> p kt m", p=128)[:, :, msl])
                    nc.sync.dma_start(
                        out=wlo[:],
                        in_=wlo_d.rearrange("(kt p) m -> p kt m", p=128)[:, :, msl])
                else:
                    wr = wpool.tile([128, KT, 128], F32R, tag="wr")
                    nc.sync.dma_start(
                        out=wr[:],
                        in_=wr_d.rearrange("(kt p) m -> p kt m", p=128)[:, :, msl].bitcast(F32R))
                for n in range(NT):
                    nsl = slice(n * 512, (n + 1) * 512)
                    pg = pgpool.tile([128, 512], F32, tag="pg")
                    seq = []
                    if L1_TERMS == 3:
                        for k in range(KT):
                            seq.append((whi[:, k, :], xThi[k][:, nsl]))
                        for k in range(KT):
                            seq.append((whi[:, k, :], xTlo[k][:, nsl]))
                        for k in range(KT):
                            seq.append((wlo[:, k, :], xThi[k][:, nsl]))
                    else:
                        for k in range(KT):
                            seq.append((wr[:, k, :], xTr[k][:, nsl]))
                    for i, (l, r) in enumerate(seq):
                        nc.tensor.matmul(pg[:], l, r,
                                         start=(i == 0), stop=(i == len(seq) - 1))
                    ev = evpool.tile([128, 512], F32, tag="ev")
                    nc.vector.tensor_copy(ev[:], pg[:])
                    nc.sync.dma_start(out=pT_d[msl, nsl], in_=ev[:])
    nc.compile()
    return nc


# ---------------------------------------------------------------- L2
def build_l2():
    nc = bacc.Bacc(name="gru_l2")
    xi_d = nc.dram_tensor("xi", [128, B, S], F32, kind="ExternalInput")
    xf_d = nc.dram_tensor("xf", [128, B, S], F32, kind="ExternalInput")
    xr_d = nc.dram_tensor("xr", [128, B, S], F32, kind="ExternalInput")
    sr_d = nc.dram_tensor("sr", [128, 128], F32, kind="ExternalInput")
    sf_d = nc.dram_tensor("sf", [128, 128], F32, kind="ExternalInput")
    sc_d = nc.dram_tensor("sc", [128, 128], F32, kind="ExternalInput")
    id_d = nc.dram_tensor("identr", [128, 128], F32, kind="ExternalInput")
    h_d = nc.dram_tensor("hT", [128, B, S], F32, kind="ExternalOutput")

    NCH = S // TC

    with TileContext(nc) as tc:
        with tc.tile_pool(name="const", bufs=1) as cpool, \
             tc.tile_pool(name="xg", bufs=2) as xpool, \
             tc.tile_pool(name="h", bufs=1) as hpool, \
             tc.tile_pool(name="scr", bufs=3) as spool, \
             tc.tile_pool(name="ps", bufs=6, space="PSUM") as ppool:

            sr = cpool.tile([128, 128], F32R, tag="sr")
            sf = cpool.tile([128, 128], F32R, tag="sf")
            sc = cpool.tile([128, 128], F32R, tag="sc")
            idr = cpool.tile([128, 128], F32R, tag="idr")
            nc.sync.dma_start(out=sr[:], in_=sr_d[:].bitcast(F32R))
            nc.sync.dma_start(out=sf[:], in_=sf_d[:].bitcast(F32R))
            nc.sync.dma_start(out=sc[:], in_=sc_d[:].bitcast(F32R))
            nc.sync.dma_start(out=idr[:], in_=id_d[:].bitcast(F32R))

            hA = hpool.tile([128, B, TC + 1], F32R, tag="hA")
            hB = hpool.tile([128, B, TC + 1], F32R, tag="hB")
            # chunk-0 boundary state: h(-1) = 0
            nc.gpsimd.memset(hA[:, :, 0:1].bitcast(F32), 0.0)
            nc.gpsimd.memset(hB[:, :, 0:1].bitcast(F32), 0.0)

            for ch in range(NCH):
                tsl = slice(ch * TC, (ch + 1) * TC)
                xi_t = xpool.tile([128, B, TC], F32R, tag="xi")
                xf_t = xpool.tile([128, B, TC], F32R, tag="xf")
                xr_t = xpool.tile([128, B, TC], F32R, tag="xr")
                nc.sync.dma_start(out=xi_t[:], in_=xi_d[:, :, tsl].bitcast(F32R))
                nc.sync.dma_start(out=xf_t[:], in_=xf_d[:, :, tsl].bitcast(F32R))
                nc.sync.dma_start(out=xr_t[:], in_=xr_d[:, :, tsl].bitcast(F32R))
                # sweep-0 reads hA = [boundary, 0, 0, ...]
                nc.gpsimd.memset(hA[:, :, 1:TC + 1].bitcast(F32), 0.0)

                for k in range(N_SWEEPS):
                    hr, hw = (hA, hB) if k % 2 == 0 else (hB, hA)
                    for b in range(B):
                        hprev = hr[:, b, 0:TC]
                        pr = ppool.tile([128, TC], F32, tag="pr")
                        nc.tensor.matmul(pr[:], idr[:], xr_t[:, b, :],
                                         start=True, stop=False)
                        nc.tensor.matmul(pr[:], sr[:], hprev,
                                         start=False, stop=True)
                        pf = ppool.tile([128, TC], F32, tag="pf")
                        nc.tensor.matmul(pf[:], idr[:], xf_t[:, b, :],
                                         start=True, stop=False)
                        nc.tensor.matmul(pf[:], sf[:], hprev,
                                         start=False, stop=True)
                        r_s = spool.tile([128, TC], F32, tag="r")
                        f_s = spool.tile([128, TC], F32, tag="f")
                        fb_s = spool.tile([128, TC], F32, tag="fb")
                        nc.scalar.activation(r_s[:], pr[:], AF.Sigmoid)
                        nc.scalar.activation(f_s[:], pf[:], AF.Sigmoid)
                        nc.scalar.activation(fb_s[:], pf[:], AF.Sigmoid, scale=-1.0)
                        rh_s = spool.tile([128, TC], F32R, tag="rh")
                        nc.vector.tensor_mul(rh_s[:], r_s[:], hprev.bitcast(F32))
                        pc = ppool.tile([128, TC], F32, tag="pc")
                        nc.tensor.matmul(pc[:], idr[:], xi_t[:, b, :],
                                         start=True, stop=False)
                        nc.tensor.matmul(pc[:], sc[:], rh_s[:],
                                         start=False, stop=True)
                        c_s = spool.tile([128, TC], F32, tag="c")
                        nc.scalar.activation(c_s[:], pc[:], AF.Tanh)
                        u_s = spool.tile([128, TC], F32, tag="u")
                        nc.vector.tensor_mul(u_s[:], fb_s[:], c_s[:])
                        nc.vector.tensor_tensor_scan(
                            hw[:, b, 1:TC + 1], f_s[:], u_s[:],
                            hw[:, b, 0:1].bitcast(F32), ALU.mult, ALU.add)

                final = hB if (N_SWEEPS - 1) % 2 == 0 else hA
                nc.sync.dma_start(out=h_d[:, :, tsl],
                                  in_=final[:, :, 1:TC + 1].bitcast(F32))
                if ch < NCH - 1:
                    nc.vector.tensor_copy(hA[:, :, 0:1],
                                          final[:, :, TC:TC + 1].bitcast(F32))
                    nc.vector.tensor_copy(hB[:, :, 0:1],
                                          final[:, :, TC:TC + 1].bitcast(F32))
    nc.compile()
    return nc


# ---------------------------------------------------------------- L3
def build_l3():
    nc = bacc.Bacc(name="gru_l3")
    h_din = nc.dram_tensor("h", [D_STATE, S], F32, kind="ExternalInput")
    g_din = nc.dram_tensor("g", [D_STATE, S], F32, kind="ExternalInput")
    if L3_TERMS == 3:
        whi_d = nc.dram_tensor("whi", [D_STATE, D_OUT], BF16, kind="ExternalInput")
        wlo_d = nc.dram_tensor("wlo", [D_STATE, D_OUT], BF16, kind="ExternalInput")
    else:
        wr_d = nc.dram_tensor("wr", [D_STATE, D_OUT], F32, kind="ExternalInput")
    o_d = nc.dram_tensor("out", [S, D_OUT], F32, kind="ExternalOutput")

    KT = D_STATE // 128   # 8
    MO = D_OUT // 128     # 8
    NT = S // 512         # 4

    with TileContext(nc) as tc:
        with tc.tile_pool(name="const", bufs=1) as cpool, \
             tc.tile_pool(name="io", bufs=3) as iopool, \
             tc.tile_pool(name="y", bufs=1) as ypool, \
             tc.tile_pool(name="w", bufs=1) as wpool, \
             tc.tile_pool(name="scr", bufs=2) as spool, \
             tc.tile_pool(name="oT", bufs=2) as opool:

            ident = cpool.tile([128, 128], F32)
            make_identity(nc, ident[:])
            ones_col = cpool.tile([128, 1], F32)
            nc.gpsimd.memset(ones_col[:], 1.0)
            ones_row = cpool.tile([1, 128], F32)
            nc.gpsimd.memset(ones_row[:], 1.0)

            if L3_TERMS == 3:
                yhi = [ypool.tile([128, S], BF16, tag=f"yhi{k}") for k in range(KT)]
                ylo = [ypool.tile([128, S], BF16, tag=f"ylo{k}") for k in range(KT)]
                whi = wpool.tile([128, KT, D_OUT], BF16, tag="whi")
                wlo = wpool.tile([128, KT, D_OUT], BF16, tag="wlo")
                nc.sync.dma_start(
                    out=whi[:], in_=whi_d.rearrange("(kt p) m -> p kt m", p=128))
                nc.sync.dma_start(
                    out=wlo[:], in_=wlo_d.rearrange("(kt p) m -> p kt m", p=128))
            else:
                yr = [ypool.tile([128, S], F32R, tag=f"yr{k}") for k in range(KT)]
                wr = wpool.tile([128, KT, D_OUT], F32R, tag="wr")
                nc.sync.dma_start(
                    out=wr[:],
                    in_=wr_d.rearrange("(kt p) m -> p kt m", p=128).bitcast(F32R))

            with tc.tile_pool(name="pssq", bufs=1, space="PSUM") as sqpool:
                psq = [sqpool.tile([1, 512], F32, tag=f"psq{n}") for n in range(NT)]
                for dt in range(KT):
                    h_t = iopool.tile([128, S], F32, tag="h")
                    g_t = iopool.tile([128, S], F32, tag="g")
                    nc.sync.dma_start(out=h_t[:], in_=h_din[dt * 128:(dt + 1) * 128, :])
                    nc.sync.dma_start(out=g_t[:], in_=g_din[dt * 128:(dt + 1) * 128, :])
                    sg = spool.tile([128, S], F32, tag="sg")
                    nc.scalar.activation(sg[:], g_t[:], AF.Silu)
                    y_t = spool.tile([128, S], F32, tag="y")
                    nc.vector.tensor_mul(y_t[:], h_t[:], sg[:])
                    if L3_TERMS == 3:
                        nc.vector.tensor_copy(yhi[dt][:], y_t[:])
                        nc.vector.tensor_sub(ylo[dt][:], y_t[:], yhi[dt][:])
                    else:
                        nc.vector.tensor_copy(yr[dt][:], y_t[:])
                    y2 = spool.tile([128, S], F32, tag="y2")
                    nc.scalar.activation(y2[:], y_t[:], AF.Square)
                    for n in range(NT):
                        nc.tensor.matmul(psq[n][:], ones_col[:],
                                         y2[:, n * 512:(n + 1) * 512],
                                         start=(dt == 0), stop=(dt == KT - 1))
                # s = 1/sqrt(sumsq/D + eps), broadcast across partitions
                s_bc = cpool.tile([128, S], F32)
                with tc.tile_pool(name="psb", bufs=2, space="PSUM") as bpool:
                    for n in range(NT):
                        sq = spool.tile([1, 512], F32, tag="sq")
                        nc.scalar.activation(sq[:], psq[n][:], AF.Sqrt,
                                             scale=1.0 / D_STATE, bias=EPS)
                        sr = spool.tile([1, 512], F32, tag="srec")
                        nc.vector.reciprocal(sr[:], sq[:])
                        pb = bpool.tile([128, 512], F32, tag="pb")
                        nc.tensor.matmul(pb[:], ones_row[:], sr[:],
                                         start=True, stop=True)
                        nc.vector.tensor_copy(s_bc[:, n * 512:(n + 1) * 512], pb[:])

            with tc.tile_pool(name="pg", bufs=2, space="PSUM") as pgpool, \
                 tc.tile_pool(name="ptr", bufs=2, space="PSUM") as ptrpool, \
                 tc.tile_pool(name="ev", bufs=2) as evpool:
                for n in range(NT):
                    nsl = slice(n * 512, (n + 1) * 512)
                    oT = opool.tile([128, 4, D_OUT], F32, tag="oT")
                    for mo in range(MO):
                        pg = pgpool.tile([128, 512], F32, tag="pg")
                        msl = slice(mo * 128, (mo + 1) * 128)
                        seq = []
                        if L3_TERMS == 3:
                            for k in range(KT):
                                seq.append((whi[:, k, msl], yhi[k][:, nsl]))
                            for k in range(KT):
                                seq.append((whi[:, k, msl], ylo[k][:, nsl]))
                            for k in range(KT):
                                seq.append((wlo[:, k, msl], yhi[k][:, nsl]))
                        else:
                            for k in range(KT):
                                seq.append((wr[:, k, msl], yr[k][:, nsl]))
                        for i, (l, r) in enumerate(seq):
                            nc.tensor.matmul(pg[:], l, r,
                                             start=(i == 0), stop=(i == len(seq) - 1))
                        ev = evpool.tile([128, 512], F32, tag="ev")
                        nc.vector.tensor_mul(ev[:], pg[:], s_bc[:, nsl])
                        for j in range(4):
                            pt = ptrpool.tile([128, 128], F32, tag="pt")
                            nc.tensor.transpose(pt[:], ev[:, j * 128:(j + 1) * 128],
                                                ident[:])
                            nc.vector.tensor_copy(oT[:, j, msl], pt[:])
                    for j in range(4):
                        nc.sync.dma_start(
                            out=o_d[n * 512 + j * 128: n * 512 + (j + 1) * 128, :],
                            in_=oT[:, j, :])
    nc.compile()
    return nc


_programs = {}


def _get_programs():
    if not _programs:
        _programs["l1"] = build_l1()
        _programs["l2"] = build_l2()
        _programs["l3"] = build_l3()
    return _programs


def kernel(x, w_in, state_weight, norm_weight, w_out):
    x = np.asarray(x, np.float32)
    w_in = np.asarray(w_in, np.float32)
    state_weight = np.asarray(state_weight, np.float32)
    norm_weight = np.asarray(norm_weight, np.float32)
    w_out = np.asarray(w_out, np.float32)

    progs = _get_programs()
    cores = list(range(N_CORES))

    # ---- L1: input projection, batch-sharded
    if L1_TERMS == 3:
        whi, wlo = _bf16_split(w_in)
        l1_ins = [{"x": np.ascontiguousarray(x[b]), "whi": whi, "wlo": wlo}
                  for b in range(B)]
    else:
        wr = _f32r_round(w_in)
        l1_ins = [{"x": np.ascontiguousarray(x[b]), "wr": wr} for b in range(B)]
    l1_res = run_bass_kernel_spmd(progs["l1"], l1_ins, cores)
    projT = [l1_res.results[b]["projT"] for b in range(B)]  # [4096, 2048] each

    # ---- L2: recurrence sweeps, head-sharded (2 heads per core)
    Wc, Wf, Wr = (state_weight[:H], state_weight[H:2 * H], state_weight[2 * H:])
    identr = np.eye(128, dtype=np.float32)
    l2_ins = []
    for c in range(N_CORES):
        rows = slice(c * 128, (c + 1) * 128)
        xi = np.stack([projT[b][rows, :] for b in range(B)], axis=1)
        xf = np.stack([projT[b][D_STATE + c * 128: D_STATE + (c + 1) * 128, :]
                       for b in range(B)], axis=1)
        xr = np.stack([projT[b][2 * D_STATE + c * 128: 2 * D_STATE + (c + 1) * 128, :]
                       for b in range(B)], axis=1)

        def blkdiag(Wg):
            m = np.zeros((128, 128), np.float32)
            m[:DH, :DH] = Wg[2 * c]
            m[DH:, DH:] = Wg[2 * c + 1]
            return _f32r_round(m)

        l2_ins.append({
            "xi": _f32r_round(np.ascontiguousarray(xi)),
            "xf": _f32r_round(np.ascontiguousarray(xf)),
            "xr": _f32r_round(np.ascontiguousarray(xr)),
            "sr": blkdiag(Wr), "sf": blkdiag(Wf), "sc": blkdiag(Wc),
            "identr": identr,
        })
    l2_res = run_bass_kernel_spmd(progs["l2"], l2_ins, cores)
    hT = [l2_res.results[c]["hT"] for c in range(N_CORES)]  # [128, B, S]

    # ---- L3: output stage, batch-sharded
    w_outp = norm_weight[:, None].astype(np.float32) * w_out
    if L3_TERMS == 3:
        whi3, wlo3 = _bf16_split(w_outp)
        wkey = {"whi": whi3, "wlo": wlo3}
    else:
        wkey = {"wr": _f32r_round(w_outp)}
    l3_ins = []
    for b in range(B):
        hb = np.concatenate([hT[c][:, b, :] for c in range(N_CORES)], axis=0)
        gb = projT[b][3 * D_STATE:, :]
        l3_ins.append({"h": np.ascontiguousarray(hb),
                       "g": np.ascontiguousarray(gb), **wkey})
    l3_res = run_bass_kernel_spmd(progs["l3"], l3_ins, cores)
    out = np.stack([l3_res.results[b]["out"] for b in range(B)], axis=0)
    return out.astype(np.float32)
